# revision 5
# baseline (speedup 1.0000x reference)
"""Trainium2 Bass kernel for the Dezfouli2019 GQL recurrent model.

Model (reference semantics, per batch element b and action a, D latent dims):
    q_t[d] = (1-phi[d]) * q_{t-1}[d] + phi[d] * rw_t * ch_t        (q_{-1} = 0.5)
    h_t[d] = (1-chi[d]) * h_{t-1}[d] + chi[d] * ch_t               (h_{-1} = 0.0)
    logits_t = sum_d beta[d] q_t[d] + sum_d kappa[d] h_t[d] + h_t C q_t
    logits_{T-1} := 0
Outputs: (logits [T,B,A], q_final [B,A,D], h_final [B,A,D]).

The scan over T is a *diagonal linear recurrence* (per-d EMA), so instead of
4096 sequential steps we compute it with block-parallel lower-triangular
matmuls on the tensor engine: time is tiled into blocks of S=127 steps; the
in-block prefix EMA is one [128x128] @ [128x512] fp32 matmul whose stationary
operand encodes phi*(1-phi)^(k-j), with the cross-block carry occupying
contraction row 127 (coefficient (1-phi)^k). The carry for the next block is
PSUM row 127 of the current block, copied by the scalar engine - a short
serial chain that overlaps with the streaming DMA, which is the real
bottleneck (memory-bound problem: ~29 MB in + ~8.4 MB out per core).

Sharding: pure data parallel over batch. B=2048 is split into 8 shards of
256; parameters are tiny and replicated. Each core's free width is
512 = 256 (batch) x 2 (actions), interleaved as b*2+a so all SBUF<->HBM
transfers are contiguous per partition.

Fast path conditions (hold for the module-init parameter values the problem
uses): phi equal across d, chi equal across d, kappa == 0, C == 0. Then
q is identical across d, logits = (beta0+beta1) * q, and h is only needed
for the final state h_{T-1} (computed as a weighted-sum matvec over the last
blocks; with chi = 0.5 contributions older than ~150 steps are exactly zero
in fp32). Any other parameter values fall back to an exact numpy port of the
reference recurrence.
"""

import numpy as np

# Problem constants (from the reference module).
N_ACTIONS = 2
D = 2
Q_INIT = 0.5
H_INIT = 0.0
T_FULL = 4096
B_FULL = 2048
N_CORES = 8
S = 127  # time steps per block (127 u-rows + 1 carry row = 128 contraction)


def _bass_mods():
    try:
        from concourse import bacc, mybir, tile
        from concourse.bass_utils import run_bass_kernel_spmd
    except ImportError:
        import sys
        for p in ("/opt/trn_rl_repo", "/root/.axon_site/_ro/trn_rl_repo"):
            if p not in sys.path:
                sys.path.insert(0, p)
        from concourse import bacc, mybir, tile
        from concourse.bass_utils import run_bass_kernel_spmd
    return bacc, mybir, tile, run_bass_kernel_spmd


def _sigmoid_f32(x):
    x = np.float32(x)
    return np.float32(1.0 / (1.0 + np.exp(-x, dtype=np.float32)))


def build_nc(T, W, n_hb, logit_scale, reps=1):
    """Build the per-core Bass program.

    T: timesteps; W: free width (= batch_shard * N_ACTIONS); n_hb: number of
    trailing time blocks whose ch values contribute to h_final; logit_scale:
    sum(beta); reps: run the whole pipeline `reps` times (idempotent - used
    only for wall-clock benchmarking).
    """
    bacc, mybir, tile, _ = _bass_mods()
    f32 = mybir.dt.float32
    NB = (T + S - 1) // S
    CW = (W // N_ACTIONS) * 7  # X row width: batch_shard * 7 input columns
    n_hb = max(1, min(n_hb, NB))

    nc = bacc.Bacc("TRN2", target_bir_lowering=False, debug=False)
    x_d = nc.dram_tensor("x", [T, CW], f32, kind="ExternalInput")
    aq_d = nc.dram_tensor("aq", [128, 128], f32, kind="ExternalInput")
    wh_d = nc.dram_tensor("wh", [128, n_hb], f32, kind="ExternalInput")
    lg_d = nc.dram_tensor("logits", [T, W], f32, kind="ExternalOutput")
    qf_d = nc.dram_tensor("qf", [1, W], f32, kind="ExternalOutput")
    hf_d = nc.dram_tensor("hf", [1, W], f32, kind="ExternalOutput")

    h_blocks = set(range(NB - n_hb, NB))

    with tile.TileContext(nc) as tc:
        with (
            tc.tile_pool(name="const", bufs=1) as constp,
            tc.tile_pool(name="xp", bufs=6) as xp,
            tc.tile_pool(name="rp", bufs=3) as rp,
            tc.tile_pool(name="lp", bufs=3) as lp,
            tc.tile_pool(name="chp", bufs=2) as chp,
            tc.tile_pool(name="outp", bufs=1) as outp,
            tc.tile_pool(name="psp", bufs=4, space="PSUM") as psp,
            tc.tile_pool(name="pshp", bufs=1, space="PSUM") as pshp,
        ):
            aq = constp.tile([128, 128], f32)
            nc.sync.dma_start(aq[:], aq_d[:])
            wh = constp.tile([128, n_hb], f32)
            nc.sync.dma_start(wh[:], wh_d[:])

            for _ in range(reps):
                hp = pshp.tile([1, W], f32)
                P_prev = None
                hmm = 0
                for kb in range(NB):
                    t0 = kb * S
                    Sk = min(S, T - t0)
                    X = xp.tile([128, CW], f32)
                    nc.sync.dma_start(X[0:Sk, :], x_d[t0:t0 + Sk, :])
                    x3 = X[:].rearrange("p (b c) -> p b c", c=7)

                    R = rp.tile([128, W], f32)
                    r3 = R[:].rearrange("p (b a) -> p b a", a=N_ACTIONS)
                    # Engine-op SBUF operands must start at partition
                    # 0/32/64/96, so the carry (contraction row 127) is
                    # installed by copying the aligned window [96:128) of the
                    # previous block's PSUM; rows 96..126 get overwritten by
                    # the u-build right after (program order => WAW dep).
                    if kb == 0:
                        nc.vector.memset(R[96:128, :], float(Q_INIT))
                    else:
                        nc.scalar.copy(R[96:128, :], P_prev[96:128, :])
                    # u = reward * choice, interleaved (b, a); row 127 keeps
                    # the carry. Two 2D-AP ops (3D APs overflow the
                    # instruction's sync-wait encoding in walrus).
                    n_u = min(Sk, S)
                    for a in range(N_ACTIONS):
                        nc.vector.tensor_mul(
                            r3[0:n_u, :, a], x3[0:n_u, :, a], x3[0:n_u, :, 2 + a]
                        )
                    if Sk < 96:
                        # Partial last block: zero the unused contraction rows
                        # (32-aligned chunks; their weights never reach the
                        # columns we read, but the sim requires them defined).
                        assert Sk % 32 == 0, Sk
                        for s in range(Sk, 96, 32):
                            nc.vector.memset(R[s:s + 32, :], 0.0)

                    P = psp.tile([128, W], f32)
                    nc.tensor.matmul(P[:], aq[:], R[:], start=True, stop=True)

                    # P[k] = q_{t0+k-1}; logits rows t0..t0+n_out-1 come from
                    # P[1:1+n_out] (t = T-1 is skipped and stays zero).
                    n_out = Sk if kb < NB - 1 else Sk - 1
                    L = lp.tile([128, W], f32)
                    nc.vector.tensor_scalar_mul(
                        L[0:1 + n_out, :], P[0:1 + n_out, :], float(logit_scale)
                    )
                    nc.sync.dma_start(lg_d[t0:t0 + n_out, :], L[1:1 + n_out, :])

                    if kb in h_blocks:
                        CH = chp.tile([128, W], f32)
                        ch3 = CH[:].rearrange("p (b a) -> p b a", a=N_ACTIONS)
                        for a in range(N_ACTIONS):
                            nc.vector.tensor_copy(ch3[0:Sk, :, a], x3[0:Sk, :, a])
                        nc.tensor.matmul(
                            hp[0:1, :], wh[0:Sk, hmm:hmm + 1], CH[0:Sk, :],
                            start=(hmm == 0), stop=(hmm == n_hb - 1),
                            skip_group_check=True,
                        )
                        hmm += 1

                    if kb == NB - 1:
                        # q_final = P[Sk] (state after step T-1); keep it at
                        # its native partition (32-aligned since Sk=32).
                        qf_sb = outp.tile([128, W], f32)
                        nc.scalar.copy(qf_sb[Sk:Sk + 1, :], P[Sk:Sk + 1, :])
                        nc.sync.dma_start(qf_d[:], qf_sb[Sk:Sk + 1, :])
                        hf_sb = outp.tile([1, W], f32)
                        nc.scalar.copy(hf_sb[0:1, :], hp[0:1, :])
                        nc.sync.dma_start(hf_d[:], hf_sb[:])

                    P_prev = P
    # bacc passes: 1-wait-per-instruction legalization (EventSemaphore
    # splitting), register allocation, nop fusion.
    nc.compile()
    return nc


def host_constants(phi, chi, T, n_hb):
    """EMA coefficient matrices, computed in float64, stored fp32.

    aq[j, k] (j<S): weight of u_{t0+j} in q_{t0+k-1}; aq[127, k]: weight of
    the carry q_{t0-1}. Column 0 is unused. wh[j, i]: weight of ch_{t0_i+j}
    in h_{T-1} for the i-th contributing block.
    """
    phi64 = float(np.float32(phi))
    alpha64 = float(np.float32(1.0) - np.float32(phi))
    aq = np.zeros((128, 128), np.float64)
    for k in range(1, 128):
        j = np.arange(k)
        aq[j, k] = phi64 * alpha64 ** ((k - 1) - j)
        aq[S, k] = alpha64 ** k
    chi64 = float(np.float32(chi))
    cbar64 = float(np.float32(1.0) - np.float32(chi))
    NB = (T + S - 1) // S
    n_hb = max(1, min(n_hb, NB))
    wh = np.zeros((128, n_hb), np.float64)
    for i, kb in enumerate(range(NB - n_hb, NB)):
        t0 = kb * S
        Sk = min(S, T - t0)
        j = np.arange(Sk)
        wh[j, i] = chi64 * cbar64 ** (T - 1 - (t0 + j))
    return aq.astype(np.float32), wh.astype(np.float32)


_NC_CACHE = {}


def _get_nc(T, W, n_hb, logit_scale, reps=1):
    key = (T, W, n_hb, float(logit_scale), reps)
    if key not in _NC_CACHE:
        _NC_CACHE[key] = build_nc(T, W, n_hb, logit_scale, reps=reps)
    return _NC_CACHE[key]


def make_in_maps(x, aq, wh, n_cores):
    """Per-core input maps: slice batch, flatten trailing dims."""
    T, B, CI = x.shape
    bs = B // n_cores
    maps = []
    for c in range(n_cores):
        xs = np.ascontiguousarray(x[:, c * bs:(c + 1) * bs, :]).reshape(T, bs * CI)
        maps.append({"x": xs, "aq": aq, "wh": wh})
    return maps


def assemble_outputs(results, T, B, n_cores):
    bs = B // n_cores
    logits = np.concatenate(
        [r["logits"].reshape(T, bs, N_ACTIONS) for r in results], axis=1
    )
    logits[-1, :, :] = 0.0
    qf = np.concatenate(
        [np.repeat(r["qf"].reshape(bs, N_ACTIONS, 1), D, axis=2) for r in results],
        axis=0,
    )
    hf = np.concatenate(
        [np.repeat(r["hf"].reshape(bs, N_ACTIONS, 1), D, axis=2) for r in results],
        axis=0,
    )
    return logits, qf, hf


def _numpy_reference(x, phi_logit, chi_logit, beta, kappa, C):
    """Exact fp32 port of the reference recurrence (general fallback)."""
    T, B, _ = x.shape
    A = N_ACTIONS
    actions = x[:, :, :A]
    rewards = x[:, :, A:2 * A]
    phi = 1.0 / (1.0 + np.exp(-phi_logit.astype(np.float32)))  # [D,1]
    chi = 1.0 / (1.0 + np.exp(-chi_logit.astype(np.float32)))
    phi = phi.T.astype(np.float32)  # [1, D]
    chi = chi.T.astype(np.float32)
    beta_v = beta[:, 0].astype(np.float32)
    kappa_v = kappa[:, 0].astype(np.float32)
    C = C.astype(np.float32)
    q = np.full((B, A, D), Q_INIT, np.float32)
    h = np.full((B, A, D), H_INIT, np.float32)
    logits = np.zeros((T, B, A), np.float32)
    one = np.float32(1.0)
    for t in range(T):
        ch = actions[t][:, :, None]
        rw = rewards[t][:, :, None]
        q = (one - phi) * q + phi * rw * ch
        h = (one - chi) * h + chi * ch
        q_w = q @ beta_v
        h_w = h @ kappa_v
        inter = np.einsum("bad,de,bae->ba", h, C, q)
        logits[t] = q_w + h_w + inter
    logits[-1] = 0.0
    return logits, q, h


def kernel(**inputs):
    x = np.asarray(inputs["inputs"], dtype=np.float32)
    phi_logit = np.asarray(inputs["phi_logit"], dtype=np.float32)
    chi_logit = np.asarray(inputs["chi_logit"], dtype=np.float32)
    beta = np.asarray(inputs["beta"], dtype=np.float32)
    kappa = np.asarray(inputs["kappa"], dtype=np.float32)
    C = np.asarray(inputs["C"], dtype=np.float32)

    T, B, CI = x.shape
    fast = (
        T == T_FULL and B == B_FULL and CI == 2 * N_ACTIONS + 3
        and phi_logit.shape == (D, 1)
        and phi_logit[0, 0] == phi_logit[1, 0]
        and chi_logit[0, 0] == chi_logit[1, 0]
        and not np.any(kappa) and not np.any(C)
    )
    if not fast:
        return _numpy_reference(x, phi_logit, chi_logit, beta, kappa, C)

    _, _, _, run_bass_kernel_spmd = _bass_mods()
    phi = _sigmoid_f32(phi_logit[0, 0])
    chi = _sigmoid_f32(chi_logit[0, 0])
    logit_scale = float(np.float32(beta[0, 0]) + np.float32(beta[1, 0]))
    NB = (T + S - 1) // S
    # chi_logit == 0 => cbar = 0.5: contributions to h_{T-1} older than the
    # last two blocks (>=159 steps back) underflow to exactly 0 in fp32.
    n_hb = 2 if chi_logit[0, 0] == 0.0 else NB

    W = (B // N_CORES) * N_ACTIONS
    aq, wh = host_constants(phi, chi, T, n_hb)
    nc = _get_nc(T, W, n_hb, logit_scale)
    in_maps = make_in_maps(x, aq, wh, N_CORES)
    res = run_bass_kernel_spmd(nc, in_maps, list(range(N_CORES)))
    return assemble_outputs(res.results, T, B, N_CORES)


# revision 6
# speedup vs baseline: 8.3218x; 8.3218x over previous
"""Trainium2 Bass kernel for the Dezfouli2019 GQL recurrent model.

Model (reference semantics, per batch element b and action a, D latent dims):
    q_t[d] = (1-phi[d]) * q_{t-1}[d] + phi[d] * rw_t * ch_t        (q_{-1} = 0.5)
    h_t[d] = (1-chi[d]) * h_{t-1}[d] + chi[d] * ch_t               (h_{-1} = 0.0)
    logits_t = sum_d beta[d] q_t[d] + sum_d kappa[d] h_t[d] + h_t C q_t
    logits_{T-1} := 0
Outputs: (logits [T,B,A], q_final [B,A,D], h_final [B,A,D]).

The scan over T is a *diagonal linear recurrence* (per-d EMA), so instead of
4096 sequential steps we compute it with block-parallel lower-triangular
matmuls on the tensor engine: time is tiled into blocks of S=127 steps; the
in-block prefix EMA is one [128x128] @ [128x512] fp32 matmul whose stationary
operand encodes phi*(1-phi)^(k-j), with the cross-block carry occupying
contraction row 127 (coefficient (1-phi)^k). The carry for the next block is
PSUM row 127 of the current block, copied by the scalar engine - a short
serial chain that overlaps with the streaming DMA, which is the real
bottleneck (memory-bound problem: ~29 MB in + ~8.4 MB out per core).

Sharding: pure data parallel over batch. B=2048 is split into 8 shards of
256; parameters are tiny and replicated. Each core's free width is
512 = 256 (batch) x 2 (actions), interleaved as b*2+a so all SBUF<->HBM
transfers are contiguous per partition.

Fast path conditions (hold for the module-init parameter values the problem
uses): phi equal across d, chi equal across d, kappa == 0, C == 0. Then
q is identical across d, logits = (beta0+beta1) * q, and h is only needed
for the final state h_{T-1} (computed as a weighted-sum matvec over the last
blocks; with chi = 0.5 contributions older than ~150 steps are exactly zero
in fp32). Any other parameter values fall back to an exact numpy port of the
reference recurrence.
"""

import numpy as np

# Problem constants (from the reference module).
N_ACTIONS = 2
D = 2
Q_INIT = 0.5
H_INIT = 0.0
T_FULL = 4096
B_FULL = 2048
N_CORES = 8
S = 127  # time steps per block (127 u-rows + 1 carry row = 128 contraction)


def _bass_mods():
    try:
        from concourse import bacc, mybir, tile
        from concourse.bass_utils import run_bass_kernel_spmd
    except ImportError:
        import sys
        for p in ("/opt/trn_rl_repo", "/root/.axon_site/_ro/trn_rl_repo"):
            if p not in sys.path:
                sys.path.insert(0, p)
        from concourse import bacc, mybir, tile
        from concourse.bass_utils import run_bass_kernel_spmd
    return bacc, mybir, tile, run_bass_kernel_spmd


def _sigmoid_f32(x):
    x = np.float32(x)
    return np.float32(1.0 / (1.0 + np.exp(-x, dtype=np.float32)))


def build_nc(T, W, n_hb, logit_scale, reps=1):
    """Build the per-core Bass program.

    T: timesteps; W: free width (= batch_shard * N_ACTIONS); n_hb: number of
    trailing time blocks whose ch values contribute to h_final; logit_scale:
    sum(beta); reps: run the whole pipeline `reps` times (idempotent - used
    only for wall-clock benchmarking).
    """
    bacc, mybir, tile, _ = _bass_mods()
    f32 = mybir.dt.float32
    NB = (T + S - 1) // S
    CW = (W // N_ACTIONS) * 7  # X row width: batch_shard * 7 input columns
    n_hb = max(1, min(n_hb, NB))

    nc = bacc.Bacc("TRN2", target_bir_lowering=False, debug=False)
    x_d = nc.dram_tensor("x", [T, CW], f32, kind="ExternalInput")
    aq_d = nc.dram_tensor("aq", [128, 128], f32, kind="ExternalInput")
    wh_d = nc.dram_tensor("wh", [128, n_hb], f32, kind="ExternalInput")
    lg_d = nc.dram_tensor("logits", [T, W], f32, kind="ExternalOutput")
    qf_d = nc.dram_tensor("qf", [1, W], f32, kind="ExternalOutput")
    hf_d = nc.dram_tensor("hf", [1, W], f32, kind="ExternalOutput")

    h_blocks = set(range(NB - n_hb, NB))

    with tile.TileContext(nc) as tc:
        LAG = 3  # blocks between logits eviction and its store (see below)
        with (
            tc.tile_pool(name="const", bufs=1) as constp,
            tc.tile_pool(name="xp", bufs=6) as xp,
            tc.tile_pool(name="rp", bufs=3) as rp,
            tc.tile_pool(name="lp", bufs=LAG + 2) as lp,
            tc.tile_pool(name="chp", bufs=2) as chp,
            tc.tile_pool(name="outp", bufs=1) as outp,
            tc.tile_pool(name="psp", bufs=4, space="PSUM") as psp,
            tc.tile_pool(name="pshp", bufs=1, space="PSUM") as pshp,
        ):
            aq = constp.tile([128, 128], f32)
            nc.sync.dma_start(aq[:], aq_d[:])
            wh = constp.tile([128, n_hb], f32)
            nc.sync.dma_start(wh[:], wh_d[:])

            for _ in range(reps):
                hp = pshp.tile([1, W], f32)
                P_prev = None
                hmm = 0
                pend = []  # delayed logits stores
                for kb in range(NB):
                    t0 = kb * S
                    Sk = min(S, T - t0)
                    X = xp.tile([128, CW], f32)
                    # SWDGE (gpsimd) split into 32-partition chunks: HWDGE
                    # runs this transfer at single-SDMA-engine rate (~30
                    # GB/s); 4 SWDGE chunks measure ~360 GB/s.
                    for p0 in range(0, 128, 32):
                        p1 = min(p0 + 32, Sk)
                        if p0 < Sk:
                            nc.gpsimd.dma_start(X[p0:p1, :], x_d[t0 + p0:t0 + p1, :])
                    x3 = X[:].rearrange("p (b c) -> p b c", c=7)

                    R = rp.tile([128, W], f32)
                    r3 = R[:].rearrange("p (b a) -> p b a", a=N_ACTIONS)
                    # Engine-op SBUF operands must start at partition
                    # 0/32/64/96, so the carry (contraction row 127) is
                    # installed by copying the aligned window [96:128) of the
                    # previous block's PSUM; rows 96..126 get overwritten by
                    # the u-build right after (program order => WAW dep).
                    if kb == 0:
                        nc.vector.memset(R[96:128, :], float(Q_INIT))
                    else:
                        nc.scalar.copy(R[96:128, :], P_prev[96:128, :])
                    # u = reward * choice, interleaved (b, a); row 127 keeps
                    # the carry. Two 2D-AP ops (3D APs overflow the
                    # instruction's sync-wait encoding in walrus).
                    n_u = min(Sk, S)
                    for a in range(N_ACTIONS):
                        nc.vector.tensor_mul(
                            r3[0:n_u, :, a], x3[0:n_u, :, a], x3[0:n_u, :, 2 + a]
                        )
                    if Sk < 96:
                        # Partial last block: zero the unused contraction rows
                        # (32-aligned chunks; their weights never reach the
                        # columns we read, but the sim requires them defined).
                        assert Sk % 32 == 0, Sk
                        for s in range(Sk, 96, 32):
                            nc.vector.memset(R[s:s + 32, :], 0.0)

                    P = psp.tile([128, W], f32)
                    nc.tensor.matmul(P[:], aq[:], R[:], start=True, stop=True)

                    # P[k] = q_{t0+k-1}; logits rows t0..t0+n_out-1 come from
                    # P[1:1+n_out] (t = T-1 is skipped and stays zero).
                    n_out = Sk if kb < NB - 1 else Sk - 1
                    L = lp.tile([128, W], f32)
                    nc.vector.tensor_scalar_mul(
                        L[0:1 + n_out, :], P[0:1 + n_out, :], float(logit_scale)
                    )
                    # The single SWDGE queue is FIFO: a store issued right
                    # after its eviction stalls Q7 descriptor emission (and
                    # with it the next blocks' loads) until the eviction
                    # completes. Delay each store by LAG blocks so its
                    # dependency is already met when Q7 reaches it.
                    pend.append((t0, n_out, L))
                    if len(pend) > LAG:
                        t02, n2, L2 = pend.pop(0)
                        nc.gpsimd.dma_start(lg_d[t02:t02 + n2, :], L2[1:1 + n2, :])

                    if kb in h_blocks:
                        CH = chp.tile([128, W], f32)
                        ch3 = CH[:].rearrange("p (b a) -> p b a", a=N_ACTIONS)
                        for a in range(N_ACTIONS):
                            nc.vector.tensor_copy(ch3[0:Sk, :, a], x3[0:Sk, :, a])
                        nc.tensor.matmul(
                            hp[0:1, :], wh[0:Sk, hmm:hmm + 1], CH[0:Sk, :],
                            start=(hmm == 0), stop=(hmm == n_hb - 1),
                            skip_group_check=True,
                        )
                        hmm += 1

                    if kb == NB - 1:
                        # q_final = P[Sk] (state after step T-1); keep it at
                        # its native partition (32-aligned since Sk=32).
                        qf_sb = outp.tile([128, W], f32)
                        nc.scalar.copy(qf_sb[Sk:Sk + 1, :], P[Sk:Sk + 1, :])
                        nc.sync.dma_start(qf_d[:], qf_sb[Sk:Sk + 1, :])
                        hf_sb = outp.tile([1, W], f32)
                        nc.scalar.copy(hf_sb[0:1, :], hp[0:1, :])
                        nc.sync.dma_start(hf_d[:], hf_sb[:])

                    P_prev = P
                for t02, n2, L2 in pend:
                    nc.gpsimd.dma_start(lg_d[t02:t02 + n2, :], L2[1:1 + n2, :])
    # bacc passes: 1-wait-per-instruction legalization (EventSemaphore
    # splitting), register allocation, nop fusion.
    nc.compile()
    return nc


def host_constants(phi, chi, T, n_hb):
    """EMA coefficient matrices, computed in float64, stored fp32.

    aq[j, k] (j<S): weight of u_{t0+j} in q_{t0+k-1}; aq[127, k]: weight of
    the carry q_{t0-1}. Column 0 is unused. wh[j, i]: weight of ch_{t0_i+j}
    in h_{T-1} for the i-th contributing block.
    """
    phi64 = float(np.float32(phi))
    alpha64 = float(np.float32(1.0) - np.float32(phi))
    aq = np.zeros((128, 128), np.float64)
    for k in range(1, 128):
        j = np.arange(k)
        aq[j, k] = phi64 * alpha64 ** ((k - 1) - j)
        aq[S, k] = alpha64 ** k
    chi64 = float(np.float32(chi))
    cbar64 = float(np.float32(1.0) - np.float32(chi))
    NB = (T + S - 1) // S
    n_hb = max(1, min(n_hb, NB))
    wh = np.zeros((128, n_hb), np.float64)
    for i, kb in enumerate(range(NB - n_hb, NB)):
        t0 = kb * S
        Sk = min(S, T - t0)
        j = np.arange(Sk)
        wh[j, i] = chi64 * cbar64 ** (T - 1 - (t0 + j))
    return aq.astype(np.float32), wh.astype(np.float32)


_NC_CACHE = {}


def _get_nc(T, W, n_hb, logit_scale, reps=1):
    key = (T, W, n_hb, float(logit_scale), reps)
    if key not in _NC_CACHE:
        _NC_CACHE[key] = build_nc(T, W, n_hb, logit_scale, reps=reps)
    return _NC_CACHE[key]


def make_in_maps(x, aq, wh, n_cores):
    """Per-core input maps: slice batch, flatten trailing dims."""
    T, B, CI = x.shape
    bs = B // n_cores
    maps = []
    for c in range(n_cores):
        xs = np.ascontiguousarray(x[:, c * bs:(c + 1) * bs, :]).reshape(T, bs * CI)
        maps.append({"x": xs, "aq": aq, "wh": wh})
    return maps


def assemble_outputs(results, T, B, n_cores):
    bs = B // n_cores
    logits = np.concatenate(
        [r["logits"].reshape(T, bs, N_ACTIONS) for r in results], axis=1
    )
    logits[-1, :, :] = 0.0
    qf = np.concatenate(
        [np.repeat(r["qf"].reshape(bs, N_ACTIONS, 1), D, axis=2) for r in results],
        axis=0,
    )
    hf = np.concatenate(
        [np.repeat(r["hf"].reshape(bs, N_ACTIONS, 1), D, axis=2) for r in results],
        axis=0,
    )
    return logits, qf, hf


def _numpy_reference(x, phi_logit, chi_logit, beta, kappa, C):
    """Exact fp32 port of the reference recurrence (general fallback)."""
    T, B, _ = x.shape
    A = N_ACTIONS
    actions = x[:, :, :A]
    rewards = x[:, :, A:2 * A]
    phi = 1.0 / (1.0 + np.exp(-phi_logit.astype(np.float32)))  # [D,1]
    chi = 1.0 / (1.0 + np.exp(-chi_logit.astype(np.float32)))
    phi = phi.T.astype(np.float32)  # [1, D]
    chi = chi.T.astype(np.float32)
    beta_v = beta[:, 0].astype(np.float32)
    kappa_v = kappa[:, 0].astype(np.float32)
    C = C.astype(np.float32)
    q = np.full((B, A, D), Q_INIT, np.float32)
    h = np.full((B, A, D), H_INIT, np.float32)
    logits = np.zeros((T, B, A), np.float32)
    one = np.float32(1.0)
    for t in range(T):
        ch = actions[t][:, :, None]
        rw = rewards[t][:, :, None]
        q = (one - phi) * q + phi * rw * ch
        h = (one - chi) * h + chi * ch
        q_w = q @ beta_v
        h_w = h @ kappa_v
        inter = np.einsum("bad,de,bae->ba", h, C, q)
        logits[t] = q_w + h_w + inter
    logits[-1] = 0.0
    return logits, q, h


def kernel(**inputs):
    x = np.asarray(inputs["inputs"], dtype=np.float32)
    phi_logit = np.asarray(inputs["phi_logit"], dtype=np.float32)
    chi_logit = np.asarray(inputs["chi_logit"], dtype=np.float32)
    beta = np.asarray(inputs["beta"], dtype=np.float32)
    kappa = np.asarray(inputs["kappa"], dtype=np.float32)
    C = np.asarray(inputs["C"], dtype=np.float32)

    T, B, CI = x.shape
    fast = (
        T == T_FULL and B == B_FULL and CI == 2 * N_ACTIONS + 3
        and phi_logit.shape == (D, 1)
        and phi_logit[0, 0] == phi_logit[1, 0]
        and chi_logit[0, 0] == chi_logit[1, 0]
        and not np.any(kappa) and not np.any(C)
    )
    if not fast:
        return _numpy_reference(x, phi_logit, chi_logit, beta, kappa, C)

    _, _, _, run_bass_kernel_spmd = _bass_mods()
    phi = _sigmoid_f32(phi_logit[0, 0])
    chi = _sigmoid_f32(chi_logit[0, 0])
    logit_scale = float(np.float32(beta[0, 0]) + np.float32(beta[1, 0]))
    NB = (T + S - 1) // S
    # chi_logit == 0 => cbar = 0.5: contributions to h_{T-1} older than the
    # last two blocks (>=159 steps back) underflow to exactly 0 in fp32.
    n_hb = 2 if chi_logit[0, 0] == 0.0 else NB

    W = (B // N_CORES) * N_ACTIONS
    aq, wh = host_constants(phi, chi, T, n_hb)
    nc = _get_nc(T, W, n_hb, logit_scale)
    in_maps = make_in_maps(x, aq, wh, N_CORES)
    res = run_bass_kernel_spmd(nc, in_maps, list(range(N_CORES)))
    return assemble_outputs(res.results, T, B, N_CORES)
